# revision 12
# baseline (speedup 1.0000x reference)
"""Trainium2 Bass kernel for nn_NumFeatureExtractor (embedding_lookup family).

Computation (reference):
    xn  = LayerNorm_F(x)                     # [B, F],  eps 1e-5
    y   = xn[:,:,None]*W + b + pos_emb[:F] + tok_emb[0]      # [B, F, H]
    out = LayerNorm_H(y) * emb_ln_g + emb_ln_b               # eps 1e-12

Shapes: B=2048, F=64, H=768.  Output [2048, 64, 768] f32 (384 MiB) -> the
kernel is output-DMA bound (~48 MiB per core across 8 cores).

Strategy (pure data parallel, batch sharded 8 ways; params replicated):
  * Host folds ln_g/ln_b into W_eff = ln_g*W, B_eff = ln_b*W + b + pos + tok.
  * Stage A (on device, per core): z = (x - mu_F)/std_F  raw-normalized x.
  * LayerNorm_H stats are ANALYTIC: with y = z*W_eff[f] + B_eff[f],
        sum_h y   = z*SW[f]  + SB[f]
        sum_h y^2 = z^2*SWW[f] + 2z*SWB[f] + SBB[f]
    where SW/SB/SWW/SWB/SBB are per-f row sums precomputed on host in f64.
    So the per-tile device work is only:
        y   = (W2 * z_col) + B2          (one DVE scalar_tensor_tensor)
        out = y*rstd_col + d_col          (one ScalarE activation Identity)
        DMA out tile -> DRAM              (393 KiB contiguous)
  * Row layout: flat row r = b*64 + f; tile t covers rows [128t, 128t+128).
    Since 128 = 2*F, partition p of every tile has f = p % 64, so W2/B2 are
    the same [128, H] tiles (W_eff stacked twice) for every t.
"""

import numpy as np

B_FULL = 2048
F = 64
H = 768
N_CORES = 8
B_LOC = B_FULL // N_CORES          # 256 batch rows per core
ROWS_LOC = B_LOC * F               # 16384 rows of H per core
N_TILES = ROWS_LOC // 128          # 128 tiles per core
EPS1 = 1e-5
EPS2 = 1e-12

_CACHE = {}
_NEFF_CACHE_DIR = "/root/.bass_neff_cache"


def _install_neff_cache():
    """The bass_exec compile path (bass2jax.neuronx_cc_hook) calls walrus
    directly with no NEFF cache (~6-20 min per compile). Wrap it with a disk
    cache keyed on the BIR bytes so repeat runs are fast."""
    import hashlib
    import os
    import shutil

    import concourse.bass2jax as b2j

    if getattr(b2j, "_ant_neff_cache_installed", False):
        return
    orig = b2j.compile_bir_kernel

    def cached_compile(bir_json, tmpdir, neff_name="file.neff"):
        h = hashlib.sha256(
            bir_json if isinstance(bir_json, bytes) else bir_json.encode()
        ).hexdigest()[:32]
        os.makedirs(_NEFF_CACHE_DIR, exist_ok=True)
        ent = os.path.join(_NEFF_CACHE_DIR, f"{h}.neff")
        if os.path.exists(ent):
            dst_dir = os.path.join(tmpdir, "sg00")
            os.makedirs(dst_dir, exist_ok=True)
            dst = os.path.join(dst_dir, neff_name)
            shutil.copy(ent, dst)
            return dst
        out = orig(bir_json, tmpdir, neff_name)
        shutil.copy(out, ent + ".tmp")
        os.replace(ent + ".tmp", ent)
        return out

    b2j.compile_bir_kernel = cached_compile
    b2j._ant_neff_cache_installed = True


def _build(identity_affine: bool, repeat: int = 1):
    """Trace + compile the Bass kernel. Returns (nc, names dict)."""
    from contextlib import ExitStack

    import concourse.bacc as bacc
    import concourse.bass as bass
    import concourse.tile as tile
    from concourse import mybir

    f32 = mybir.dt.float32
    Alu = mybir.AluOpType
    Act = mybir.ActivationFunctionType

    nc = bacc.Bacc(
        "TRN2",
        target_bir_lowering=False,
        debug=False,
        enable_asserts=False,
        num_devices=N_CORES,
    )

    x_d = nc.dram_tensor("x_loc", [B_LOC, F], f32, kind="ExternalInput")
    w_d = nc.dram_tensor("w_eff", [F, H], f32, kind="ExternalInput")
    b_d = nc.dram_tensor("b_eff", [F, H], f32, kind="ExternalInput")
    # columns: [-SW/H, -SB/H, SWW/H, 2*SWB/H, SBB/H]
    c_d = nc.dram_tensor("ln2c", [F, 5], f32, kind="ExternalInput")
    i_d = nc.dram_tensor("ident", [128, 128], f32, kind="ExternalInput")
    if not identity_affine:
        eg_d = nc.dram_tensor("emb_g", [H], f32, kind="ExternalInput")
        eb_d = nc.dram_tensor("emb_b", [H], f32, kind="ExternalInput")
    out_d = nc.dram_tensor("out_loc", [ROWS_LOC, H], f32, kind="ExternalOutput")

    with tile.TileContext(nc) as tc:
        with ExitStack() as ctx:
            singles = ctx.enter_context(tc.tile_pool(name="singles", bufs=1))
            stats = ctx.enter_context(tc.tile_pool(name="stats", bufs=1))
            psum = ctx.enter_context(tc.tile_pool(name="psum", bufs=1, space="PSUM"))
            ypool = ctx.enter_context(tc.tile_pool(name="ypool", bufs=4))
            opool = ctx.enter_context(tc.tile_pool(name="opool", bufs=6))

            # ---- constants into SBUF --------------------------------------
            w2 = singles.tile([128, H], f32)
            nc.sync.dma_start(out=w2[0:64, :], in_=w_d.ap())
            nc.sync.dma_start(out=w2[64:128, :], in_=w_d.ap())
            b2 = singles.tile([128, H], f32)
            nc.sync.dma_start(out=b2[0:64, :], in_=b_d.ap())
            nc.sync.dma_start(out=b2[64:128, :], in_=b_d.ap())
            cst = singles.tile([128, 5], f32)
            nc.sync.dma_start(out=cst[0:64, :], in_=c_d.ap())
            nc.sync.dma_start(out=cst[64:128, :], in_=c_d.ap())
            ident = singles.tile([128, 128], f32)
            nc.sync.dma_start(out=ident, in_=i_d.ap())
            if not identity_affine:
                egb = singles.tile([128, H], f32)
                nc.gpsimd.dma_start(out=egb, in_=eg_d.ap().to_broadcast([128, H]))
                ebb = singles.tile([128, H], f32)
                nc.gpsimd.dma_start(out=ebb, in_=eb_d.ap().to_broadcast([128, H]))

            eps1_t = singles.tile([128, 1], f32)
            nc.vector.memset(eps1_t, EPS1)
            eps2_t = singles.tile([128, 1], f32)
            nc.vector.memset(eps2_t, EPS2)

            # ---- stage A: z = (x - mu_F) / std_F --------------------------
            # X[p, a, f] = x[2p + a, f]; flat row r = 128p + 64a + f.
            xt = singles.tile([128, 2, F], f32)
            nc.sync.dma_start(out=xt, in_=x_d.ap().rearrange("(p a) f -> p a f", a=2))
            z_sb = singles.tile([128, 2, F], f32)
            for a in range(2):
                st6 = stats.tile([128, 6], f32, tag=f"st6_{a}")
                nc.vector.bn_stats(out=st6, in_=xt[:, a, :])
                mv = stats.tile([128, 2], f32, tag=f"mv_{a}")
                nc.vector.bn_aggr(out=mv, in_=st6)
                sd = stats.tile([128, 1], f32, tag=f"sd_{a}")
                nc.scalar.activation(
                    out=sd, in_=mv[:, 1:2], func=Act.Sqrt, bias=eps1_t, scale=1.0
                )
                r0 = stats.tile([128, 1], f32, tag=f"r0_{a}")
                nc.vector.reciprocal(out=r0, in_=sd)
                # Newton polish on rsqrt (ACT Sqrt table is low-precision):
                # rst = r0 * (1.5 - 0.5*(var+eps)*r0^2)
                v1 = stats.tile([128, 1], f32, tag=f"v1_{a}")
                nc.vector.tensor_scalar(
                    out=v1, in0=mv[:, 1:2], scalar1=EPS1, scalar2=None, op0=Alu.add
                )
                nr = stats.tile([128, 1], f32, tag=f"nr_{a}")
                nc.vector.tensor_mul(out=nr, in0=r0, in1=r0)
                nc.vector.tensor_mul(out=nr, in0=nr, in1=v1)
                nc.vector.tensor_scalar(
                    out=nr, in0=nr, scalar1=-0.5, scalar2=1.5, op0=Alu.mult,
                    op1=Alu.add,
                )
                rst = stats.tile([128, 1], f32, tag=f"rst_{a}")
                nc.vector.tensor_mul(out=rst, in0=r0, in1=nr)
                nc.vector.tensor_scalar(
                    out=z_sb[:, a, :],
                    in0=xt[:, a, :],
                    scalar1=mv[:, 0:1],
                    scalar2=rst,
                    op0=Alu.subtract,
                    op1=Alu.mult,
                )

            # ---- transpose z so tile t's scalars are a column -------------
            # ZT[p, t] = z_flat[128t + p]
            zt_ps = psum.tile([128, 128], f32)
            nc.tensor.transpose(
                zt_ps, z_sb.rearrange("p a f -> p (a f)"), ident
            )
            zt = singles.tile([128, 128], f32)
            nc.vector.tensor_copy(out=zt, in_=zt_ps)

            # ---- analytic LayerNorm_H stats (batched over all tiles) ------
            # negmu[p,t] = z*(-SW/H) + (-SB/H)
            negmu = stats.tile([128, 128], f32)
            nc.vector.tensor_scalar(
                out=negmu, in0=zt, scalar1=cst[:, 0:1], scalar2=cst[:, 1:2],
                op0=Alu.mult, op1=Alu.add,
            )
            # e2n = z^2*(SWW/H) + (z*(2SWB/H) + SBB/H) = E[y^2]
            zsq = stats.tile([128, 128], f32)
            nc.vector.tensor_mul(out=zsq, in0=zt, in1=zt)
            e2a = stats.tile([128, 128], f32)
            nc.vector.tensor_scalar(
                out=e2a, in0=zsq, scalar1=cst[:, 2:3], scalar2=None,
                op0=Alu.mult,
            )
            e2b = stats.tile([128, 128], f32)
            nc.vector.tensor_scalar(
                out=e2b, in0=zt, scalar1=cst[:, 3:4], scalar2=cst[:, 4:5],
                op0=Alu.mult, op1=Alu.add,
            )
            # var = e2a + e2b - mu^2 ; use stt: (negmu*negmu) subtracted
            e2 = stats.tile([128, 128], f32)
            nc.vector.tensor_add(out=e2, in0=e2a, in1=e2b)
            musq = stats.tile([128, 128], f32)
            nc.vector.tensor_mul(out=musq, in0=negmu, in1=negmu)
            var = stats.tile([128, 128], f32)
            nc.vector.tensor_sub(out=var, in0=e2, in1=musq)
            sdev = stats.tile([128, 128], f32)
            nc.scalar.activation(
                out=sdev, in_=var, func=Act.Sqrt, bias=eps2_t, scale=1.0
            )
            r0b = stats.tile([128, 128], f32)
            nc.vector.reciprocal(out=r0b, in_=sdev)
            # Newton polish: rstd = r0*(1.5 - 0.5*(var+eps)*r0^2)
            v2 = stats.tile([128, 128], f32)
            nc.vector.tensor_scalar(
                out=v2, in0=var, scalar1=EPS2, scalar2=None, op0=Alu.add
            )
            nrb = stats.tile([128, 128], f32)
            nc.vector.tensor_mul(out=nrb, in0=r0b, in1=r0b)
            nc.vector.tensor_mul(out=nrb, in0=nrb, in1=v2)
            nc.vector.tensor_scalar(
                out=nrb, in0=nrb, scalar1=-0.5, scalar2=1.5, op0=Alu.mult,
                op1=Alu.add,
            )
            rstd = stats.tile([128, 128], f32)
            nc.vector.tensor_mul(out=rstd, in0=r0b, in1=nrb)
            dbias = stats.tile([128, 128], f32)
            nc.vector.tensor_mul(out=dbias, in0=negmu, in1=rstd)

            # ---- main loop: one DVE + one ACT + one DMA per tile ----------
            # repeat>1 wraps the loop in a HW For_i purely for wall-clock
            # timing by differencing (output is idempotent).
            from contextlib import nullcontext

            out_ap = out_d.ap()
            rep_cm = tc.For_i(0, repeat, 1) if repeat > 1 else nullcontext()
            with rep_cm:
                _main_loop(nc, tc, ypool, opool, w2, b2, zt, rstd, dbias,
                           out_ap, identity_affine,
                           egb if not identity_affine else None,
                           ebb if not identity_affine else None)

    nc.compile()
    return nc


def _main_loop(nc, tc, ypool, opool, w2, b2, zt, rstd, dbias, out_ap,
               identity_affine, egb, ebb):
    from concourse import mybir

    f32 = mybir.dt.float32
    Alu = mybir.AluOpType
    Act = mybir.ActivationFunctionType
    for t in range(N_TILES):
        y = ypool.tile([128, H], f32, tag="y")
        nc.vector.scalar_tensor_tensor(
            out=y, in0=w2, scalar=zt[:, t : t + 1], in1=b2,
            op0=Alu.mult, op1=Alu.add,
        )
        o = opool.tile([128, H], f32, tag="o")
        nc.scalar.activation(
            out=o, in_=y, func=Act.Identity,
            bias=dbias[:, t : t + 1], scale=rstd[:, t : t + 1],
        )
        if not identity_affine:
            nc.vector.tensor_mul(out=o, in0=o, in1=egb)
            nc.gpsimd.tensor_add(out=o, in0=o, in1=ebb)
        nc.sync.dma_start(out=out_ap[t * 128 : (t + 1) * 128, :], in_=o)


def _prepare_host(inputs):
    """Fold params on host (f64) -> arrays fed to every core."""
    x = np.ascontiguousarray(np.asarray(inputs["x"], dtype=np.float32))
    ln_g = np.asarray(inputs["ln_g"], dtype=np.float64)
    ln_b = np.asarray(inputs["ln_b"], dtype=np.float64)
    W = np.asarray(inputs["W"], dtype=np.float64)
    b = np.asarray(inputs["b"], dtype=np.float64)
    pos = np.asarray(inputs["pos_emb"], dtype=np.float64)[:F]
    tok = np.asarray(inputs["tok_emb"], dtype=np.float64)[0]
    eg = np.asarray(inputs["emb_ln_g"], dtype=np.float64)
    eb = np.asarray(inputs["emb_ln_b"], dtype=np.float64)

    w_eff = ln_g[:, None] * W
    b_eff = ln_b[:, None] * W + b + pos + tok[None, :]

    identity_affine = bool(np.all(eg == 1.0) and np.all(eb == 0.0))

    sw = w_eff.sum(axis=1)
    sb = b_eff.sum(axis=1)
    sww = (w_eff * w_eff).sum(axis=1)
    swb = (w_eff * b_eff).sum(axis=1)
    sbb = (b_eff * b_eff).sum(axis=1)
    ln2c = np.stack(
        [-sw / H, -sb / H, sww / H, 2.0 * swb / H, sbb / H], axis=1
    ).astype(np.float32)

    common = {
        "w_eff": np.ascontiguousarray(w_eff.astype(np.float32)),
        "b_eff": np.ascontiguousarray(b_eff.astype(np.float32)),
        "ln2c": np.ascontiguousarray(ln2c),
        "ident": np.eye(128, dtype=np.float32),
    }
    if not identity_affine:
        common["emb_g"] = eg.astype(np.float32)
        common["emb_b"] = eb.astype(np.float32)
    return x, common, identity_affine


def _run(nc, x, common, trace=False, **kw):
    from concourse.bass_utils import run_bass_kernel_spmd

    _install_neff_cache()

    in_maps = []
    for c in range(N_CORES):
        m = dict(common)
        m["x_loc"] = np.ascontiguousarray(x[c * B_LOC : (c + 1) * B_LOC])
        in_maps.append(m)
    return run_bass_kernel_spmd(
        nc, in_maps, core_ids=list(range(N_CORES)), trace=trace, **kw
    )


def kernel(**inputs) -> np.ndarray:
    x, common, identity_affine = _prepare_host(inputs)
    key = ("k", identity_affine)
    if key not in _CACHE:
        _CACHE[key] = _build(identity_affine)
    nc = _CACHE[key]
    res = _run(nc, x, common)
    outs = [r["out_loc"] for r in res.results]
    return np.concatenate(outs, axis=0).reshape(B_FULL, F, H)


# revision 14
# speedup vs baseline: 1.0092x; 1.0092x over previous
"""Trainium2 Bass kernel for nn_NumFeatureExtractor (embedding_lookup family).

Computation (reference):
    xn  = LayerNorm_F(x)                     # [B, F],  eps 1e-5
    y   = xn[:,:,None]*W + b + pos_emb[:F] + tok_emb[0]      # [B, F, H]
    out = LayerNorm_H(y) * emb_ln_g + emb_ln_b               # eps 1e-12

Shapes: B=2048, F=64, H=768.  Output [2048, 64, 768] f32 (384 MiB) -> the
kernel is output-DMA bound (~48 MiB per core across 8 cores).

Strategy (pure data parallel, batch sharded 8 ways; params replicated):
  * Host folds ln_g/ln_b into W_eff = ln_g*W, B_eff = ln_b*W + b + pos + tok.
  * Stage A (on device, per core): z = (x - mu_F)/std_F  raw-normalized x.
  * LayerNorm_H stats are ANALYTIC: with y = z*W_eff[f] + B_eff[f],
        sum_h y   = z*SW[f]  + SB[f]
        sum_h y^2 = z^2*SWW[f] + 2z*SWB[f] + SBB[f]
    where SW/SB/SWW/SWB/SBB are per-f row sums precomputed on host in f64.
    So the per-tile device work is only:
        y   = (W2 * z_col) + B2          (one DVE scalar_tensor_tensor)
        out = y*rstd_col + d_col          (one ScalarE activation Identity)
        DMA out tile -> DRAM              (393 KiB contiguous)
  * Row layout: flat row r = b*64 + f; tile t covers rows [128t, 128t+128).
    Since 128 = 2*F, partition p of every tile has f = p % 64, so W2/B2 are
    the same [128, H] tiles (W_eff stacked twice) for every t.
"""

import numpy as np

B_FULL = 2048
F = 64
H = 768
N_CORES = 8
B_LOC = B_FULL // N_CORES          # 256 batch rows per core
ROWS_LOC = B_LOC * F               # 16384 rows of H per core
N_TILES = ROWS_LOC // 128          # 128 tiles per core
EPS1 = 1e-5
EPS2 = 1e-12

_CACHE = {}
_NEFF_CACHE_DIR = "/root/.bass_neff_cache"


def _install_neff_cache():
    """The bass_exec compile path (bass2jax.neuronx_cc_hook) calls walrus
    directly with no NEFF cache (~6-20 min per compile). Wrap it with a disk
    cache keyed on the BIR bytes so repeat runs are fast."""
    import hashlib
    import os
    import shutil

    import concourse.bass2jax as b2j

    if getattr(b2j, "_ant_neff_cache_installed", False):
        return
    orig = b2j.compile_bir_kernel

    def cached_compile(bir_json, tmpdir, neff_name="file.neff"):
        h = hashlib.sha256(
            bir_json if isinstance(bir_json, bytes) else bir_json.encode()
        ).hexdigest()[:32]
        os.makedirs(_NEFF_CACHE_DIR, exist_ok=True)
        ent = os.path.join(_NEFF_CACHE_DIR, f"{h}.neff")
        if os.path.exists(ent):
            dst_dir = os.path.join(tmpdir, "sg00")
            os.makedirs(dst_dir, exist_ok=True)
            dst = os.path.join(dst_dir, neff_name)
            shutil.copy(ent, dst)
            return dst
        out = orig(bir_json, tmpdir, neff_name)
        shutil.copy(out, ent + ".tmp")
        os.replace(ent + ".tmp", ent)
        return out

    b2j.compile_bir_kernel = cached_compile
    b2j._ant_neff_cache_installed = True


def _build(identity_affine: bool, repeat: int = 1):
    """Trace + compile the Bass kernel. Returns (nc, names dict)."""
    from contextlib import ExitStack

    import concourse.bacc as bacc
    import concourse.bass as bass
    import concourse.tile as tile
    from concourse import mybir

    f32 = mybir.dt.float32
    Alu = mybir.AluOpType
    Act = mybir.ActivationFunctionType

    nc = bacc.Bacc(
        "TRN2",
        target_bir_lowering=False,
        debug=False,
        enable_asserts=False,
        num_devices=N_CORES,
    )

    x_d = nc.dram_tensor("x_loc", [B_LOC, F], f32, kind="ExternalInput")
    w_d = nc.dram_tensor("w_eff", [F, H], f32, kind="ExternalInput")
    b_d = nc.dram_tensor("b_eff", [F, H], f32, kind="ExternalInput")
    # columns: [-SW/H, -SB/H, SWW/H, 2*SWB/H, SBB/H]
    c_d = nc.dram_tensor("ln2c", [F, 5], f32, kind="ExternalInput")
    i_d = nc.dram_tensor("ident", [128, 128], f32, kind="ExternalInput")
    if not identity_affine:
        eg_d = nc.dram_tensor("emb_g", [H], f32, kind="ExternalInput")
        eb_d = nc.dram_tensor("emb_b", [H], f32, kind="ExternalInput")
    out_d = nc.dram_tensor("out_loc", [ROWS_LOC, H], f32, kind="ExternalOutput")

    with tile.TileContext(nc) as tc:
        with ExitStack() as ctx:
            singles = ctx.enter_context(tc.tile_pool(name="singles", bufs=1))
            stats = ctx.enter_context(tc.tile_pool(name="stats", bufs=1))
            psum = ctx.enter_context(tc.tile_pool(name="psum", bufs=1, space="PSUM"))
            ypool = ctx.enter_context(tc.tile_pool(name="ypool", bufs=4))
            opool = ctx.enter_context(tc.tile_pool(name="opool", bufs=6))

            # ---- constants into SBUF --------------------------------------
            w2 = singles.tile([128, H], f32)
            nc.sync.dma_start(out=w2[0:64, :], in_=w_d.ap())
            nc.sync.dma_start(out=w2[64:128, :], in_=w_d.ap())
            b2 = singles.tile([128, H], f32)
            nc.sync.dma_start(out=b2[0:64, :], in_=b_d.ap())
            nc.sync.dma_start(out=b2[64:128, :], in_=b_d.ap())
            cst = singles.tile([128, 5], f32)
            nc.sync.dma_start(out=cst[0:64, :], in_=c_d.ap())
            nc.sync.dma_start(out=cst[64:128, :], in_=c_d.ap())
            ident = singles.tile([128, 128], f32)
            nc.sync.dma_start(out=ident, in_=i_d.ap())
            if not identity_affine:
                egb = singles.tile([128, H], f32)
                nc.gpsimd.dma_start(out=egb, in_=eg_d.ap().to_broadcast([128, H]))
                ebb = singles.tile([128, H], f32)
                nc.gpsimd.dma_start(out=ebb, in_=eb_d.ap().to_broadcast([128, H]))

            eps1_t = singles.tile([128, 1], f32)
            nc.vector.memset(eps1_t, EPS1)
            eps2_t = singles.tile([128, 1], f32)
            nc.vector.memset(eps2_t, EPS2)

            # ---- stage A: z = (x - mu_F) / std_F --------------------------
            # X[p, a, f] = x[2p + a, f]; flat row r = 128p + 64a + f.
            xt = singles.tile([128, 2, F], f32)
            nc.sync.dma_start(out=xt, in_=x_d.ap().rearrange("(p a) f -> p a f", a=2))
            z_sb = singles.tile([128, 2, F], f32)
            for a in range(2):
                st6 = stats.tile([128, 6], f32, tag=f"st6_{a}")
                nc.vector.bn_stats(out=st6, in_=xt[:, a, :])
                mv = stats.tile([128, 2], f32, tag=f"mv_{a}")
                nc.vector.bn_aggr(out=mv, in_=st6)
                sd = stats.tile([128, 1], f32, tag=f"sd_{a}")
                nc.scalar.activation(
                    out=sd, in_=mv[:, 1:2], func=Act.Sqrt, bias=eps1_t, scale=1.0
                )
                r0 = stats.tile([128, 1], f32, tag=f"r0_{a}")
                nc.vector.reciprocal(out=r0, in_=sd)
                # Newton polish on rsqrt (ACT Sqrt table is low-precision):
                # rst = r0 * (1.5 - 0.5*(var+eps)*r0^2)
                v1 = stats.tile([128, 1], f32, tag=f"v1_{a}")
                nc.vector.tensor_scalar(
                    out=v1, in0=mv[:, 1:2], scalar1=EPS1, scalar2=None, op0=Alu.add
                )
                nr = stats.tile([128, 1], f32, tag=f"nr_{a}")
                nc.vector.tensor_mul(out=nr, in0=r0, in1=r0)
                nc.vector.tensor_mul(out=nr, in0=nr, in1=v1)
                nc.vector.tensor_scalar(
                    out=nr, in0=nr, scalar1=-0.5, scalar2=1.5, op0=Alu.mult,
                    op1=Alu.add,
                )
                rst = stats.tile([128, 1], f32, tag=f"rst_{a}")
                nc.vector.tensor_mul(out=rst, in0=r0, in1=nr)
                nc.vector.tensor_scalar(
                    out=z_sb[:, a, :],
                    in0=xt[:, a, :],
                    scalar1=mv[:, 0:1],
                    scalar2=rst,
                    op0=Alu.subtract,
                    op1=Alu.mult,
                )

            # ---- transpose z so tile t's scalars are a column -------------
            # ZT[p, t] = z_flat[128t + p]
            zt_ps = psum.tile([128, 128], f32)
            nc.tensor.transpose(
                zt_ps, z_sb.rearrange("p a f -> p (a f)"), ident
            )
            zt = singles.tile([128, 128], f32)
            nc.vector.tensor_copy(out=zt, in_=zt_ps)

            # ---- analytic LayerNorm_H stats (batched over all tiles) ------
            # negmu[p,t] = z*(-SW/H) + (-SB/H)
            negmu = stats.tile([128, 128], f32)
            nc.vector.tensor_scalar(
                out=negmu, in0=zt, scalar1=cst[:, 0:1], scalar2=cst[:, 1:2],
                op0=Alu.mult, op1=Alu.add,
            )
            # e2n = z^2*(SWW/H) + (z*(2SWB/H) + SBB/H) = E[y^2]
            zsq = stats.tile([128, 128], f32)
            nc.vector.tensor_mul(out=zsq, in0=zt, in1=zt)
            e2a = stats.tile([128, 128], f32)
            nc.vector.tensor_scalar(
                out=e2a, in0=zsq, scalar1=cst[:, 2:3], scalar2=None,
                op0=Alu.mult,
            )
            e2b = stats.tile([128, 128], f32)
            nc.vector.tensor_scalar(
                out=e2b, in0=zt, scalar1=cst[:, 3:4], scalar2=cst[:, 4:5],
                op0=Alu.mult, op1=Alu.add,
            )
            # var = e2a + e2b - mu^2 ; use stt: (negmu*negmu) subtracted
            e2 = stats.tile([128, 128], f32)
            nc.vector.tensor_add(out=e2, in0=e2a, in1=e2b)
            musq = stats.tile([128, 128], f32)
            nc.vector.tensor_mul(out=musq, in0=negmu, in1=negmu)
            var = stats.tile([128, 128], f32)
            nc.vector.tensor_sub(out=var, in0=e2, in1=musq)
            sdev = stats.tile([128, 128], f32)
            nc.scalar.activation(
                out=sdev, in_=var, func=Act.Sqrt, bias=eps2_t, scale=1.0
            )
            r0b = stats.tile([128, 128], f32)
            nc.vector.reciprocal(out=r0b, in_=sdev)
            # Newton polish: rstd = r0*(1.5 - 0.5*(var+eps)*r0^2)
            v2 = stats.tile([128, 128], f32)
            nc.vector.tensor_scalar(
                out=v2, in0=var, scalar1=EPS2, scalar2=None, op0=Alu.add
            )
            nrb = stats.tile([128, 128], f32)
            nc.vector.tensor_mul(out=nrb, in0=r0b, in1=r0b)
            nc.vector.tensor_mul(out=nrb, in0=nrb, in1=v2)
            nc.vector.tensor_scalar(
                out=nrb, in0=nrb, scalar1=-0.5, scalar2=1.5, op0=Alu.mult,
                op1=Alu.add,
            )
            rstd = stats.tile([128, 128], f32)
            nc.vector.tensor_mul(out=rstd, in0=r0b, in1=nrb)
            dbias = stats.tile([128, 128], f32)
            nc.vector.tensor_mul(out=dbias, in0=negmu, in1=rstd)

            # ---- main loop: one DVE + one ACT + one DMA per tile ----------
            # repeat>1 wraps the loop in a HW For_i purely for wall-clock
            # timing by differencing (output is idempotent).
            from contextlib import nullcontext

            out_ap = out_d.ap()
            rep_cm = tc.For_i(0, repeat, 1) if repeat > 1 else nullcontext()
            with rep_cm:
                _main_loop(nc, tc, ypool, opool, w2, b2, zt, rstd, dbias,
                           out_ap, identity_affine,
                           egb if not identity_affine else None,
                           ebb if not identity_affine else None)

    nc.compile()
    return nc


def _main_loop(nc, tc, ypool, opool, w2, b2, zt, rstd, dbias, out_ap,
               identity_affine, egb, ebb):
    from concourse import mybir

    f32 = mybir.dt.float32
    Alu = mybir.AluOpType
    Act = mybir.ActivationFunctionType
    for t in range(N_TILES):
        y = ypool.tile([128, H], f32, tag="y")
        nc.vector.scalar_tensor_tensor(
            out=y, in0=w2, scalar=zt[:, t : t + 1], in1=b2,
            op0=Alu.mult, op1=Alu.add,
        )
        o = opool.tile([128, H], f32, tag="o")
        nc.scalar.activation(
            out=o, in_=y, func=Act.Identity,
            bias=dbias[:, t : t + 1], scale=rstd[:, t : t + 1],
        )
        if not identity_affine:
            nc.vector.tensor_mul(out=o, in0=o, in1=egb)
            nc.gpsimd.tensor_add(out=o, in0=o, in1=ebb)
        nc.sync.dma_start(out=out_ap[t * 128 : (t + 1) * 128, :], in_=o)


def _prepare_host(inputs):
    """Fold params on host (f64) -> arrays fed to every core."""
    x = np.ascontiguousarray(np.asarray(inputs["x"], dtype=np.float32))
    ln_g = np.asarray(inputs["ln_g"], dtype=np.float64)
    ln_b = np.asarray(inputs["ln_b"], dtype=np.float64)
    W = np.asarray(inputs["W"], dtype=np.float64)
    b = np.asarray(inputs["b"], dtype=np.float64)
    pos = np.asarray(inputs["pos_emb"], dtype=np.float64)[:F]
    tok = np.asarray(inputs["tok_emb"], dtype=np.float64)[0]
    eg = np.asarray(inputs["emb_ln_g"], dtype=np.float64)
    eb = np.asarray(inputs["emb_ln_b"], dtype=np.float64)

    w_eff = ln_g[:, None] * W
    b_eff = ln_b[:, None] * W + b + pos + tok[None, :]

    identity_affine = bool(np.all(eg == 1.0) and np.all(eb == 0.0))

    sw = w_eff.sum(axis=1)
    sb = b_eff.sum(axis=1)
    sww = (w_eff * w_eff).sum(axis=1)
    swb = (w_eff * b_eff).sum(axis=1)
    sbb = (b_eff * b_eff).sum(axis=1)
    ln2c = np.stack(
        [-sw / H, -sb / H, sww / H, 2.0 * swb / H, sbb / H], axis=1
    ).astype(np.float32)

    common = {
        "w_eff": np.ascontiguousarray(w_eff.astype(np.float32)),
        "b_eff": np.ascontiguousarray(b_eff.astype(np.float32)),
        "ln2c": np.ascontiguousarray(ln2c),
        "ident": np.eye(128, dtype=np.float32),
    }
    if not identity_affine:
        common["emb_g"] = eg.astype(np.float32)
        common["emb_b"] = eb.astype(np.float32)
    return x, common, identity_affine


def _run(nc, x, common, trace=False, **kw):
    from concourse.bass_utils import run_bass_kernel_spmd

    _install_neff_cache()

    in_maps = []
    for c in range(N_CORES):
        m = dict(common)
        m["x_loc"] = np.ascontiguousarray(x[c * B_LOC : (c + 1) * B_LOC])
        in_maps.append(m)
    return run_bass_kernel_spmd(
        nc, in_maps, core_ids=list(range(N_CORES)), trace=trace, **kw
    )


def kernel(**inputs) -> np.ndarray:
    x, common, identity_affine = _prepare_host(inputs)
    key = ("k", identity_affine)
    if key not in _CACHE:
        _CACHE[key] = _build(identity_affine)
    nc = _CACHE[key]
    res = _run(nc, x, common)
    outs = [r["out_loc"] for r in res.results]
    return np.concatenate(outs, axis=0).reshape(B_FULL, F, H)
